# revision 10
# baseline (speedup 1.0000x reference)
"""Trainium2 Bass kernel for DensityGCNProcessor.

Model: 2-layer GCN over a per-sample kNN graph built from 1-D density values
(K=4 nearest by |density_i - density_j|), symmetric deg^-1/2 normalization on
target indegree, relu after each layer.

Strategy
--------
kNN in a 1-D metric means: after sorting nodes by density, every node's 4
nearest neighbours lie within +/-4 sorted positions. So the whole aggregation
matrix becomes a 9-diagonal *banded* matrix in sorted order. The device kernel:

  1. transposes X^T [Cin, N] tiles on the TensorEngine and indirect-DMA
     scatters node rows into a DRAM scratch in *sorted* order (per-core rank
     window of 2048 nodes + halo),
  2. computes A1 = Band @ X_s with small banded matmuls (TensorEngine,
     float32r = full-precision fp32 at 1 cycle/row),
  3. H^T = relu(W1^T A1^T + b1) dense matmuls (channel-major),
  4. T2^T = W2^T H^T, transposed back to node-major,
  5. out = relu(Band @ T2 + b2), indirect-DMA scattered to original node order.

Host does only O(N log N) index math on the 16 KB density array: argsort, band
weights w9[r, o] (including exact reference tie-breaking by (dist, orig index),
which also reproduces the reference's duplicate-density self-target quirk), and
expands them into the per-tile band matrices.

Sharding: 8 cores = 4 batches x 2 rank-halves. Core c handles batch c//2,
sorted ranks [ (c%2)*2048, (c%2)*2048+2048 ).
"""

import numpy as np

# ---------------------------------------------------------------- constants
B = 4
CIN = 256
CHID = 512
COUT = 256
H = W = 64
N = H * W            # 4096 nodes per batch
KNN = 4
BAND = 4             # kNN lies within +/-4 sorted positions
HALF = N // 2        # 2048 ranks per core
NT1 = 17             # A1/H/T2 tiles (rows r0-4 .. r0+2172)
NT2 = 16             # output tiles  (rows r0   .. r0+2048)
XS_ROWS = HALF + 136  # 2184 scratch rows, local row l <-> rank r0 - 8 + l
SENTINEL = 1 << 20

_COMPILED = {}


# ---------------------------------------------------------------- host graph
def _build_band_weights(d_flat):
    """order [N], w9 [N, 9] f32: out_s[r] = sum_o w9[r, o+4] * g_s[r+o]."""
    order = np.argsort(d_flat, kind="stable")
    d_s = d_flat[order]

    offs = np.arange(-BAND, BAND + 1)
    ridx = np.arange(N)[:, None] + offs[None, :]
    valid = (ridx >= 0) & (ridx < N)
    ridx_c = np.clip(ridx, 0, N - 1)
    c = np.abs(d_s[ridx_c] - d_s[:, None]).astype(np.float32)
    c = np.where(valid, c, np.float32(np.inf))
    cand_j = np.where(valid, order[ridx_c], N)

    # reference = stable argsort over the full row: ties by smaller orig index.
    sel = np.lexsort((cand_j, c), axis=1)
    tgt_s = np.take_along_axis(ridx_c, sel[:, 1:KNN + 1], axis=1).reshape(-1)
    src_s = np.repeat(np.arange(N), KNN)

    deg = np.ones(N, dtype=np.float32)
    np.add.at(deg, tgt_s, np.float32(1.0))
    dinv = (np.float32(1.0) / np.sqrt(deg)).astype(np.float32)

    m = np.zeros((N, 9), dtype=np.float32)
    np.add.at(m, (tgt_s, src_s - tgt_s + BAND), np.float32(1.0))
    m[:, BAND] += 1.0  # self loops

    ro = np.arange(N)[:, None] + offs[None, :]
    rov = (ro >= 0) & (ro < N)
    w9 = m * dinv[:, None] * dinv[np.clip(ro, 0, N - 1)] * rov
    return order.astype(np.int32), w9.astype(np.float32)


def _host_graph(density_maps):
    """Per-core index/band tensors. Returns list of 8 dicts."""
    per_core = []
    for b in range(B):
        d = np.asarray(density_maps[b]).reshape(N).astype(np.float32)
        order, w9g = _build_band_weights(d)
        rank = np.empty(N, dtype=np.int64)
        rank[order] = np.arange(N)
        for half in range(2):
            r0 = half * HALF

            # scatter index: orig node j (= col of xT) -> local scratch row
            loc = rank - (r0 - 8)
            scat = np.where((loc >= 0) & (loc < XS_ROWS), loc, SENTINEL)
            scat_idx = scat.reshape(N // 128, 128).T.astype(np.int32).copy()  # [128, 32]

            # w9 rows for this core's window, zero outside usable range
            # w9_dev[i] = w9 at rank (r0 - 4 + i), i in [0, NT1*128)
            w9_dev = np.zeros((NT1 * 128, 9), dtype=np.float32)
            g = np.arange(NT1 * 128) + (r0 - 4)
            ok = (g >= 0) & (g < N) & (g < r0 + HALF + 4)
            w9_dev[ok] = w9g[g[ok]]

            # band matrices bandT[k, q, r]: k<17 -> L1 tile (out rows r0-4+128k+r),
            # k>=17 -> L2 tile (out rows r0+128(k-17)+r). value = w9row[q - r].
            bandT = np.zeros((NT1 + NT2, 136, 128), dtype=np.float32)
            qq = np.arange(136)[:, None]          # window position
            rr = np.arange(128)[None, :]          # out row within tile
            dd = qq - rr                          # w9 column (o + 4)
            okd = (dd >= 0) & (dd < 9)
            dd_c = np.clip(dd, 0, 8)
            rr_b = np.broadcast_to(rr, (136, 128))
            for k in range(NT1 + NT2):
                base = 128 * k if k < NT1 else 4 + 128 * (k - NT1)
                rows = w9_dev[base + np.arange(128)]          # [128, 9]
                bandT[k] = np.where(okd, rows[rr_b, dd_c], 0.0)

            # output scatter: (p, t') -> orig index of rank r0 + 128 t' + p
            out_idx = order[r0 + (np.arange(NT2)[None, :] * 128 + np.arange(128)[:, None])].astype(np.int32).copy()

            per_core.append(dict(scat_idx=scat_idx,
                                 bandT=np.ascontiguousarray(bandT.transpose(1, 0, 2)),
                                 out_idx=out_idx, order=order, rank=rank))
    return per_core


# ---------------------------------------------------------------- device IR
def build_nc():
    import concourse.bass as bass
    import concourse.bacc as bacc
    import concourse.mybir as mybir
    from concourse.tile import TileContext

    F32 = mybir.dt.float32
    F32R = mybir.dt.float32r
    I32 = mybir.dt.int32
    NR = NT1 + NT2

    nc = bacc.Bacc()
    xT = nc.dram_tensor("xT", [CIN, N], F32R, kind="ExternalInput")
    w1 = nc.dram_tensor("w1", [CIN, CHID], F32R, kind="ExternalInput")
    w2 = nc.dram_tensor("w2", [CHID, COUT], F32R, kind="ExternalInput")
    b1 = nc.dram_tensor("b1", [CHID], F32, kind="ExternalInput")
    b2row = nc.dram_tensor("b2row", [1, COUT], F32R, kind="ExternalInput")
    ones1 = nc.dram_tensor("ones1", [1, 128], F32R, kind="ExternalInput")
    ident = nc.dram_tensor("ident", [128, 128], F32R, kind="ExternalInput")
    bandT = nc.dram_tensor("bandT", [136, NR, 128], F32R, kind="ExternalInput")
    scat_idx = nc.dram_tensor("scat_idx", [128, N // 128], I32, kind="ExternalInput")
    out_idx = nc.dram_tensor("out_idx", [128, NT2], I32, kind="ExternalInput")
    out_nodes = nc.dram_tensor("out_nodes", [N, COUT], F32, kind="ExternalOutput")
    xs = nc.dram_tensor("xs", [XS_ROWS, CIN], F32R, kind="Internal")

    NJ = N // 128  # 32 node-column tiles of xT

    with TileContext(nc) as tc:
        with (
            tc.tile_pool(name="const", bufs=1) as cpool,
            tc.tile_pool(name="big", bufs=1) as big,
            tc.tile_pool(name="stream", bufs=3) as sp,
            tc.tile_pool(name="psum", bufs=2, space="PSUM") as pp,
        ):
            ident_sb = cpool.tile([128, 128], F32R)
            nc.sync.dma_start(ident_sb, ident[:, :])
            ones_sb = cpool.tile([1, 128], F32R)
            nc.sync.dma_start(ones_sb, ones1[:, :])
            b2_sb = cpool.tile([1, COUT], F32R)
            nc.sync.dma_start(b2_sb, b2row[:, :])
            zero_sb = cpool.tile([128, CIN], F32)
            nc.gpsimd.memset(zero_sb, 0.0)

            w1_sb = cpool.tile([128, 2, CHID], F32R)   # [k-part, k-chunk, m]
            nc.sync.dma_start(w1_sb, w1.rearrange("(c p) m -> p c m", p=128))
            w2_sb = cpool.tile([128, 4, COUT], F32R)
            nc.sync.dma_start(w2_sb, w2.rearrange("(c p) m -> p c m", p=128))
            b1_sb = cpool.tile([128, 4], F32)
            nc.sync.dma_start(b1_sb, b1.rearrange("(c p) -> p c", p=128))
            scat_sb = cpool.tile([128, NJ], I32)
            nc.sync.dma_start(scat_sb, scat_idx[:, :])
            oidx_sb = cpool.tile([128, NT2], I32)
            nc.sync.dma_start(oidx_sb, out_idx[:, :])

            # all band matrices in two DMAs: [q-part, region, r]
            bandA_sb = cpool.tile([128, NR, 128], F32R)
            nc.sync.dma_start(bandA_sb, bandT[0:128, :, :])
            bandB_sb = cpool.tile([8, NR, 128], F32R)
            nc.sync.dma_start(bandB_sb, bandT[128:136, :, :])

            # ---------------- phase X: transpose X^T -> node-major, scatter sorted
            # zero rows the scatter may skip (rank outside [0, N))
            nc.sync.dma_start(xs[0:8, :].bitcast(F32), zero_sb[0:8, :])
            nc.sync.dma_start(xs[HALF + 8:XS_ROWS, :].bitcast(F32), zero_sb[:, :])
            xnode = big.tile([128, NJ, CIN], F32R)  # 4 MB
            for jh in range(NJ // 4):
                xt_sb = sp.tile([128, 512], F32R, tag="xt")
                nc.sync.dma_start(xt_sb, xT[0:128, 512 * jh:512 * (jh + 1)])
                xt_sb2 = sp.tile([128, 512], F32R, tag="xt2")
                nc.sync.dma_start(xt_sb2, xT[128:256, 512 * jh:512 * (jh + 1)])
                for j4 in range(4):
                    jt = 4 * jh + j4
                    tp = pp.tile([128, 256], F32R, tag="tp", space="PSUM")
                    nc.tensor.transpose(tp[:, 0:128], xt_sb[:, 128 * j4:128 * (j4 + 1)], ident_sb)
                    nc.tensor.transpose(tp[:, 128:256], xt_sb2[:, 128 * j4:128 * (j4 + 1)], ident_sb)
                    nc.vector.tensor_copy(xnode[:, jt, :], tp)
                    nc.gpsimd.indirect_dma_start(
                        out=xs[:, :],
                        out_offset=bass.IndirectOffsetOnAxis(ap=scat_sb[:, jt:jt + 1], axis=0),
                        in_=xnode[:, jt, :],
                        in_offset=None,
                        bounds_check=XS_ROWS - 1,
                        oob_is_err=False,
                    )

            # ---------------- L1 aggregation: A1 = Band1 @ X_s (node-major psum),
            # then transpose to A1^T (cin-major) for the dense matmul.
            a1T = big.tile([128, 2, NT1 * 128], F32R)   # A1^T, cin-chunk major
            prev_rhs = None
            rhs_tiles = []
            for t in range(NT1 + 1):
                if t < NT1:
                    r = sp.tile([128, CIN], F32R, tag="rhs0", bufs=4)
                    nc.sync.dma_start(r, xs[128 * t:128 * t + 128, :])
                else:
                    r = sp.tile([8, CIN], F32R, tag="rhs1")
                    nc.sync.dma_start(r, xs[128 * NT1:128 * NT1 + 8, :])
                rhs_tiles.append(r)
            for t in range(NT1):
                psA = pp.tile([128, CIN], F32, tag="agg", space="PSUM")
                nc.tensor.matmul(psA, lhsT=bandA_sb[:, t, :], rhs=rhs_tiles[t],
                                 start=True, stop=False)
                nc.tensor.matmul(psA, lhsT=bandB_sb[:, t, :],
                                 rhs=rhs_tiles[t + 1][0:8, :],
                                 start=False, stop=True)
                a1_sb = sp.tile([128, CIN], F32R, tag="a1")
                nc.vector.tensor_copy(a1_sb, psA)
                for cb in range(2):
                    tpa = pp.tile([128, 128], F32R, tag="tp", space="PSUM")
                    nc.tensor.transpose(tpa, a1_sb[:, 128 * cb:128 * (cb + 1)], ident_sb)
                    nc.vector.tensor_copy(a1T[:, cb, 128 * t:128 * t + 128], tpa)

            # ---------------- L1 dense: H^T = relu(W1^T A1^T + b1)  (chid-major)
            NODES = NT1 * 128
            blocks = [(i, min(i + 448, NODES)) for i in range(0, NODES, 448)]
            hT = big.tile([128, 4, NODES], F32R)
            for lo, hi in blocks:
                for mb in range(4):
                    psH = pp.tile([128, 448], F32, tag="dense", space="PSUM")
                    for kb in range(2):
                        nc.tensor.matmul(
                            psH[:, 0:hi - lo],
                            lhsT=w1_sb[:, kb, 128 * mb:128 * (mb + 1)],
                            rhs=a1T[:, kb, lo:hi],
                            start=(kb == 0), stop=(kb == 1))
                    nc.scalar.activation(
                        hT[:, mb, lo:hi], psH[:, 0:hi - lo],
                        mybir.ActivationFunctionType.Relu,
                        bias=b1_sb[:, mb:mb + 1], scale=1.0)

            # ---------------- L2 dense: T2 = H W2, node-major directly
            # lhsT = H^T slice [chid-chunk, 128 nodes], rhs = W2 chunk
            t2n = big.tile([128, NT1, COUT], F32R)
            for t in range(NT1):
                psT = pp.tile([128, COUT], F32, tag="agg", space="PSUM")
                for kb in range(4):
                    nc.tensor.matmul(
                        psT,
                        lhsT=hT[:, kb, 128 * t:128 * t + 128],
                        rhs=w2_sb[:, kb, :],
                        start=(kb == 0), stop=(kb == 3))
                nc.scalar.activation(t2n[:, t, :], psT,
                                     mybir.ActivationFunctionType.Copy)

            # ---------------- L2 aggregation + b2 (as K=1 matmul) + relu + scatter
            out_all = big.tile([128, NT2, COUT], F32)
            for t in range(NT2):
                psO = pp.tile([128, COUT], F32, tag="agg", space="PSUM")
                nc.tensor.matmul(psO, lhsT=bandA_sb[:, NT1 + t, :],
                                 rhs=t2n[:, t, :], start=True, stop=False)
                nc.tensor.matmul(psO, lhsT=bandB_sb[:, NT1 + t, :],
                                 rhs=t2n[0:8, t + 1, :], start=False, stop=False)
                nc.tensor.matmul(psO, lhsT=ones_sb, rhs=b2_sb,
                                 start=False, stop=True)
                nc.scalar.activation(out_all[:, t, :], psO,
                                     mybir.ActivationFunctionType.Relu)
                nc.gpsimd.indirect_dma_start(
                    out=out_nodes[:, :],
                    out_offset=bass.IndirectOffsetOnAxis(ap=oidx_sb[:, t:t + 1], axis=0),
                    in_=out_all[:, t, :],
                    in_offset=None,
                    bounds_check=N - 1,
                    oob_is_err=False,
                )

    nc.compile()
    return nc


def _round_f32r(a):
    bits = np.ascontiguousarray(a, dtype=np.float32).view(np.uint32)
    r = ((bits.astype(np.uint64) + 0x800) & np.uint64(0xFFFFF000)).astype(np.uint32)
    return r.view(np.float32)


def make_in_maps(density_maps, feature_maps, W1, b1, W2, b2):
    graph = _host_graph(density_maps)
    fm = np.ascontiguousarray(np.asarray(feature_maps, dtype=np.float32))
    W1 = np.ascontiguousarray(np.asarray(W1, dtype=np.float32))
    W2 = np.ascontiguousarray(np.asarray(W2, dtype=np.float32))
    b1 = np.ascontiguousarray(np.asarray(b1, dtype=np.float32))
    b2r = np.broadcast_to(np.asarray(b2, dtype=np.float32), (128, COUT)).copy()
    in_maps = []
    for c in range(8):
        g = graph[c]
        in_maps.append({
            "xT": fm[c // 2].reshape(CIN, N),
            "w1": _round_f32r(W1), "w2": _round_f32r(W2), "b1": b1,
            "b2row": _round_f32r(b2r[0:1]), "ones1": np.ones((1, 128), np.float32),
            "ident": np.eye(128, dtype=np.float32),
            "bandT": _round_f32r(g["bandT"]), "scat_idx": g["scat_idx"], "out_idx": g["out_idx"],
        })
    return in_maps, graph


def kernel(density_maps, feature_maps, W1, b1, W2, b2):
    from concourse.bass_utils import run_bass_kernel_spmd

    if "nc" not in _COMPILED:
        _COMPILED["nc"] = build_nc()
    nc = _COMPILED["nc"]

    in_maps, graph = make_in_maps(density_maps, feature_maps, W1, b1, W2, b2)
    res = run_bass_kernel_spmd(nc, in_maps, core_ids=list(range(8)))

    out = np.empty((B, N, COUT), dtype=np.float32)
    for b in range(B):
        o0 = res.results[2 * b]["out_nodes"]
        o1 = res.results[2 * b + 1]["out_nodes"]
        mask = (graph[2 * b]["rank"] < HALF)[:, None]
        out[b] = np.where(mask, o0, o1)
    return np.ascontiguousarray(
        out.reshape(B, H, W, COUT).transpose(0, 3, 1, 2)).astype(np.float32)


# revision 13
# speedup vs baseline: 1.1907x; 1.1907x over previous
"""Trainium2 Bass kernel for DensityGCNProcessor.

Model: 2-layer GCN over a per-sample kNN graph built from 1-D density values
(K=4 nearest by |density_i - density_j|), symmetric deg^-1/2 normalization on
target indegree, relu after each layer.

Strategy
--------
kNN in a 1-D metric means: after sorting nodes by density, every node's 4
nearest neighbours lie within +/-4 sorted positions. So the whole aggregation
matrix becomes a 9-diagonal *banded* matrix in sorted order. The device kernel:

  1. transposes X^T [Cin, N] tiles on the TensorEngine and indirect-DMA
     scatters node rows into a DRAM scratch in *sorted* order (per-core rank
     window of 2048 nodes + halo),
  2. computes A1 = Band @ X_s with small banded matmuls (TensorEngine,
     float32r = full-precision fp32 at 1 cycle/row),
  3. H^T = relu(W1^T A1^T + b1) dense matmuls (channel-major),
  4. T2^T = W2^T H^T, transposed back to node-major,
  5. out = relu(Band @ T2 + b2), indirect-DMA scattered to original node order.

Host does only O(N log N) index math on the 16 KB density array: argsort, band
weights w9[r, o] (including exact reference tie-breaking by (dist, orig index),
which also reproduces the reference's duplicate-density self-target quirk), and
expands them into the per-tile band matrices.

Sharding: 8 cores = 4 batches x 2 rank-halves. Core c handles batch c//2,
sorted ranks [ (c%2)*2048, (c%2)*2048+2048 ).
"""

import numpy as np

# ---------------------------------------------------------------- constants
B = 4
CIN = 256
CHID = 512
COUT = 256
H = W = 64
N = H * W            # 4096 nodes per batch
KNN = 4
BAND = 4             # kNN lies within +/-4 sorted positions
HALF = N // 2        # 2048 ranks per core
NT1 = 17             # A1/H/T2 tiles (rows r0-4 .. r0+2172)
NT2 = 16             # output tiles  (rows r0   .. r0+2048)
GATH_ROWS = (NT1 + 1) * 128  # 2304 gathered window rows (rank r0 - 8 + i)

_COMPILED = {}


# ---------------------------------------------------------------- host graph
def _build_band_weights(d_flat):
    """order [N], w9 [N, 9] f32: out_s[r] = sum_o w9[r, o+4] * g_s[r+o]."""
    order = np.argsort(d_flat, kind="stable")
    d_s = d_flat[order]

    offs = np.arange(-BAND, BAND + 1)
    ridx = np.arange(N)[:, None] + offs[None, :]
    valid = (ridx >= 0) & (ridx < N)
    ridx_c = np.clip(ridx, 0, N - 1)
    c = np.abs(d_s[ridx_c] - d_s[:, None]).astype(np.float32)
    c = np.where(valid, c, np.float32(np.inf))
    cand_j = np.where(valid, order[ridx_c], N)

    # reference = stable argsort over the full row: ties by smaller orig index.
    sel = np.lexsort((cand_j, c), axis=1)
    tgt_s = np.take_along_axis(ridx_c, sel[:, 1:KNN + 1], axis=1).reshape(-1)
    src_s = np.repeat(np.arange(N), KNN)

    deg = np.ones(N, dtype=np.float32)
    np.add.at(deg, tgt_s, np.float32(1.0))
    dinv = (np.float32(1.0) / np.sqrt(deg)).astype(np.float32)

    m = np.zeros((N, 9), dtype=np.float32)
    np.add.at(m, (tgt_s, src_s - tgt_s + BAND), np.float32(1.0))
    m[:, BAND] += 1.0  # self loops

    ro = np.arange(N)[:, None] + offs[None, :]
    rov = (ro >= 0) & (ro < N)
    w9 = m * dinv[:, None] * dinv[np.clip(ro, 0, N - 1)] * rov
    return order.astype(np.int32), w9.astype(np.float32)


def _host_graph(density_maps):
    """Per-core index/band tensors. Returns list of 8 dicts."""
    per_core = []
    for b in range(B):
        d = np.asarray(density_maps[b]).reshape(N).astype(np.float32)
        order, w9g = _build_band_weights(d)
        rank = np.empty(N, dtype=np.int64)
        rank[order] = np.arange(N)
        for half in range(2):
            r0 = half * HALF

            # gather index: local window row i (rank r0 - 8 + i) -> orig node.
            # Out-of-range ranks clip to node 0 (finite data; w9 rows are 0 there).
            gi = np.arange(GATH_ROWS) + (r0 - 8)
            gsrc = np.where((gi >= 0) & (gi < N), order[np.clip(gi, 0, N - 1)], 0)
            gidx = np.tile(gsrc.reshape(GATH_ROWS // 16, 16).T.astype(np.int16), (8, 1)).copy()  # [128, 144]

            # w9 rows for this core's window, zero outside usable range
            # w9_dev[i] = w9 at rank (r0 - 4 + i), i in [0, NT1*128)
            w9_dev = np.zeros((NT1 * 128, 9), dtype=np.float32)
            g = np.arange(NT1 * 128) + (r0 - 4)
            ok = (g >= 0) & (g < N) & (g < r0 + HALF + 4)
            w9_dev[ok] = w9g[g[ok]]

            # band matrices bandT[k, q, r]: k<17 -> L1 tile (out rows r0-4+128k+r),
            # k>=17 -> L2 tile (out rows r0+128(k-17)+r). value = w9row[q - r].
            bandT = np.zeros((NT1 + NT2, 136, 128), dtype=np.float32)
            qq = np.arange(136)[:, None]          # window position
            rr = np.arange(128)[None, :]          # out row within tile
            dd = qq - rr                          # w9 column (o + 4)
            okd = (dd >= 0) & (dd < 9)
            dd_c = np.clip(dd, 0, 8)
            rr_b = np.broadcast_to(rr, (136, 128))
            for k in range(NT1 + NT2):
                base = 128 * k if k < NT1 else 4 + 128 * (k - NT1)
                rows = w9_dev[base + np.arange(128)]          # [128, 9]
                bandT[k] = np.where(okd, rows[rr_b, dd_c], 0.0)

            # output scatter: flat i (rank r0 + i) -> orig node index
            osrc = order[r0 + np.arange(NT2 * 128)]
            oidx = np.tile(osrc.reshape(NT2 * 128 // 16, 16).T.astype(np.int16), (8, 1)).copy()  # [128, 128]

            per_core.append(dict(gidx=gidx, oidx=oidx,
                                 bandT=np.ascontiguousarray(bandT.transpose(1, 0, 2)),
                                 order=order, rank=rank))
    return per_core


# ---------------------------------------------------------------- device IR
def build_nc():
    import concourse.bass as bass
    import concourse.bacc as bacc
    import concourse.mybir as mybir
    from concourse.tile import TileContext

    F32 = mybir.dt.float32
    F32R = mybir.dt.float32r
    I32 = mybir.dt.int32
    I16 = mybir.dt.int16
    NR = NT1 + NT2

    nc = bacc.Bacc()
    xT = nc.dram_tensor("xT", [CIN, N], F32R, kind="ExternalInput")
    w1 = nc.dram_tensor("w1", [CIN, CHID], F32R, kind="ExternalInput")
    w2 = nc.dram_tensor("w2", [CHID, COUT], F32R, kind="ExternalInput")
    b1 = nc.dram_tensor("b1", [CHID], F32, kind="ExternalInput")
    b2row = nc.dram_tensor("b2row", [1, COUT], F32R, kind="ExternalInput")
    ones1 = nc.dram_tensor("ones1", [1, 128], F32R, kind="ExternalInput")
    ident = nc.dram_tensor("ident", [128, 128], F32R, kind="ExternalInput")
    bandT = nc.dram_tensor("bandT", [136, NR, 128], F32R, kind="ExternalInput")
    gidx = nc.dram_tensor("gidx", [128, GATH_ROWS // 16], I16, kind="ExternalInput")
    oidx = nc.dram_tensor("oidx", [128, NT2 * 128 // 16], I16, kind="ExternalInput")
    out_nodes = nc.dram_tensor("out_nodes", [N, COUT], F32, kind="ExternalOutput")
    xpose = nc.dram_tensor("xpose", [N, CIN], F32R, kind="Internal")

    NJ = N // 128  # 32 node-column tiles of xT

    with TileContext(nc) as tc:
        with (
            tc.tile_pool(name="const", bufs=1) as cpool,
            tc.tile_pool(name="big", bufs=1) as big,
            tc.tile_pool(name="stream", bufs=3) as sp,
            tc.tile_pool(name="psum", bufs=2, space="PSUM") as pp,
        ):
            ident_sb = cpool.tile([128, 128], F32R)
            nc.sync.dma_start(ident_sb, ident[:, :])
            ones_sb = cpool.tile([1, 128], F32R)
            nc.sync.dma_start(ones_sb, ones1[:, :])
            b2_sb = cpool.tile([1, COUT], F32R)
            nc.sync.dma_start(b2_sb, b2row[:, :])
            zero_sb = cpool.tile([128, CIN], F32)
            nc.gpsimd.memset(zero_sb, 0.0)

            w1_sb = cpool.tile([128, 2, CHID], F32R)   # [k-part, k-chunk, m]
            nc.scalar.dma_start(w1_sb, w1.rearrange("(c p) m -> p c m", p=128))
            w2_sb = cpool.tile([128, 4, COUT], F32R)
            nc.scalar.dma_start(w2_sb, w2.rearrange("(c p) m -> p c m", p=128))
            b1_sb = cpool.tile([128, 4], F32)
            nc.scalar.dma_start(b1_sb, b1.rearrange("(c p) -> p c", p=128))
            gidx_sb = cpool.tile([128, GATH_ROWS // 16], I16)
            nc.scalar.dma_start(gidx_sb, gidx[:, :])
            oidx_sb = cpool.tile([128, NT2 * 128 // 16], I16)
            nc.scalar.dma_start(oidx_sb, oidx[:, :])

            # all band matrices in two DMAs: [q-part, region, r]
            bandA_sb = cpool.tile([128, NR, 128], F32R)
            nc.scalar.dma_start(bandA_sb, bandT[0:128, :, :])
            bandB_sb = cpool.tile([8, NR, 128], F32R)
            nc.scalar.dma_start(bandB_sb, bandT[128:136, :, :])

            # ---------------- zero the output accumulator (scatter-add target)
            zero_big = cpool.tile([128, 1024], F32)
            nc.gpsimd.memset(zero_big, 0.0)
            for r in range(0, N, 512):
                nc.sync.dma_start(
                    out_nodes[r:r + 512, :].rearrange("(a b) c -> a (b c)", b=4),
                    zero_big[:, :])

            # ---------------- phase X: transpose X^T tiles into node-major DRAM,
            # then one dma_gather pulls the sorted window into SBUF.
            for jh in range(NJ // 4):
                xt_sb = sp.tile([128, 512], F32R, tag="xt")
                nc.sync.dma_start(xt_sb, xT[0:128, 512 * jh:512 * (jh + 1)])
                xt_sb2 = sp.tile([128, 512], F32R, tag="xt2")
                nc.sync.dma_start(xt_sb2, xT[128:256, 512 * jh:512 * (jh + 1)])
                for j4 in range(4):
                    jt = 4 * jh + j4
                    tp = pp.tile([128, 256], F32R, tag="tp", space="PSUM")
                    nc.tensor.transpose(tp[:, 0:128], xt_sb[:, 128 * j4:128 * (j4 + 1)], ident_sb)
                    nc.tensor.transpose(tp[:, 128:256], xt_sb2[:, 128 * j4:128 * (j4 + 1)], ident_sb)
                    xn = sp.tile([128, CIN], F32R, tag="xn")
                    nc.vector.tensor_copy(xn, tp)
                    nc.sync.dma_start(xpose[128 * jt:128 * (jt + 1), :], xn)

            gath = big.tile([128, NT1 + 1, CIN], F32R)
            nc.gpsimd.dma_gather(gath[:, :, :], xpose[:, :], gidx_sb[:, :],
                                 GATH_ROWS, GATH_ROWS, CIN, single_packet=False)

            # ---------------- L1 aggregation: A1 = Band1 @ X_s (node-major psum),
            # then transpose to A1^T (cin-major) for the dense matmul.
            a1T = big.tile([128, 2, NT1 * 128], F32R)   # A1^T, cin-chunk major
            for t in range(NT1):
                psA = pp.tile([128, CIN], F32, tag="agg", space="PSUM")
                nc.tensor.matmul(psA, lhsT=bandA_sb[:, t, :], rhs=gath[:, t, :],
                                 start=True, stop=False)
                nc.tensor.matmul(psA, lhsT=bandB_sb[:, t, :],
                                 rhs=gath[0:8, t + 1, :],
                                 start=False, stop=True)
                a1_sb = sp.tile([128, CIN], F32R, tag="a1")
                nc.vector.tensor_copy(a1_sb, psA)
                for cb in range(2):
                    tpa = pp.tile([128, 128], F32R, tag="tp", space="PSUM")
                    nc.tensor.transpose(tpa, a1_sb[:, 128 * cb:128 * (cb + 1)], ident_sb)
                    nc.vector.tensor_copy(a1T[:, cb, 128 * t:128 * t + 128], tpa)

            # ---------------- L1 dense: H^T = relu(W1^T A1^T + b1)  (chid-major)
            NODES = NT1 * 128
            blocks = [(i, min(i + 448, NODES)) for i in range(0, NODES, 448)]
            hT = big.tile([128, 4, NODES], F32R)
            for lo, hi in blocks:
                for mb in range(4):
                    psH = pp.tile([128, 448], F32, tag="dense", space="PSUM")
                    for kb in range(2):
                        nc.tensor.matmul(
                            psH[:, 0:hi - lo],
                            lhsT=w1_sb[:, kb, 128 * mb:128 * (mb + 1)],
                            rhs=a1T[:, kb, lo:hi],
                            start=(kb == 0), stop=(kb == 1))
                    nc.scalar.activation(
                        hT[:, mb, lo:hi], psH[:, 0:hi - lo],
                        mybir.ActivationFunctionType.Relu,
                        bias=b1_sb[:, mb:mb + 1], scale=1.0)

            # ---------------- L2 dense: T2 = H W2, node-major directly
            # lhsT = H^T slice [chid-chunk, 128 nodes], rhs = W2 chunk
            t2n = big.tile([128, NT1, COUT], F32R)
            for t in range(NT1):
                psT = pp.tile([128, COUT], F32, tag="agg", space="PSUM")
                for kb in range(4):
                    nc.tensor.matmul(
                        psT,
                        lhsT=hT[:, kb, 128 * t:128 * t + 128],
                        rhs=w2_sb[:, kb, :],
                        start=(kb == 0), stop=(kb == 3))
                nc.scalar.activation(t2n[:, t, :], psT,
                                     mybir.ActivationFunctionType.Copy)

            # ---------------- L2 aggregation + b2 (as K=1 matmul) + relu + scatter
            out_all = big.tile([128, NT2, COUT], F32)
            for t in range(NT2):
                psO = pp.tile([128, COUT], F32, tag="agg", space="PSUM")
                nc.tensor.matmul(psO, lhsT=bandA_sb[:, NT1 + t, :],
                                 rhs=t2n[:, t, :], start=True, stop=False)
                nc.tensor.matmul(psO, lhsT=bandB_sb[:, NT1 + t, :],
                                 rhs=t2n[0:8, t + 1, :], start=False, stop=False)
                nc.tensor.matmul(psO, lhsT=ones_sb, rhs=b2_sb,
                                 start=False, stop=True)
                nc.scalar.activation(out_all[:, t, :], psO,
                                     mybir.ActivationFunctionType.Relu)

            nc.gpsimd.dma_scatter_add(out_nodes[:, :], out_all[:, :, :],
                                      oidx_sb[:, :], NT2 * 128, NT2 * 128, COUT,
                                      single_packet=False)

    nc.compile()
    return nc


def _round_f32r(a):
    bits = np.ascontiguousarray(a, dtype=np.float32).view(np.uint32)
    r = ((bits.astype(np.uint64) + 0x800) & np.uint64(0xFFFFF000)).astype(np.uint32)
    return r.view(np.float32)


def make_in_maps(density_maps, feature_maps, W1, b1, W2, b2):
    graph = _host_graph(density_maps)
    fm = np.ascontiguousarray(np.asarray(feature_maps, dtype=np.float32))
    W1 = np.ascontiguousarray(np.asarray(W1, dtype=np.float32))
    W2 = np.ascontiguousarray(np.asarray(W2, dtype=np.float32))
    b1 = np.ascontiguousarray(np.asarray(b1, dtype=np.float32))
    b2r = np.broadcast_to(np.asarray(b2, dtype=np.float32), (128, COUT)).copy()
    in_maps = []
    for c in range(8):
        g = graph[c]
        in_maps.append({
            "xT": fm[c // 2].reshape(CIN, N),
            "w1": _round_f32r(W1), "w2": _round_f32r(W2), "b1": b1,
            "b2row": _round_f32r(b2r[0:1]), "ones1": np.ones((1, 128), np.float32),
            "ident": np.eye(128, dtype=np.float32),
            "bandT": _round_f32r(g["bandT"]), "gidx": g["gidx"], "oidx": g["oidx"],
        })
    return in_maps, graph


def kernel(density_maps, feature_maps, W1, b1, W2, b2):
    from concourse.bass_utils import run_bass_kernel_spmd

    if "nc" not in _COMPILED:
        _COMPILED["nc"] = build_nc()
    nc = _COMPILED["nc"]

    in_maps, graph = make_in_maps(density_maps, feature_maps, W1, b1, W2, b2)
    res = run_bass_kernel_spmd(nc, in_maps, core_ids=list(range(8)))

    out = np.empty((B, N, COUT), dtype=np.float32)
    for b in range(B):
        o0 = res.results[2 * b]["out_nodes"]
        o1 = res.results[2 * b + 1]["out_nodes"]
        mask = (graph[2 * b]["rank"] < HALF)[:, None]
        out[b] = np.where(mask, o0, o1)
    return np.ascontiguousarray(
        out.reshape(B, H, W, COUT).transpose(0, 3, 1, 2)).astype(np.float32)


# revision 14
# speedup vs baseline: 1.2951x; 1.0876x over previous
"""Trainium2 Bass kernel for DensityGCNProcessor.

Model: 2-layer GCN over a per-sample kNN graph built from 1-D density values
(K=4 nearest by |density_i - density_j|), symmetric deg^-1/2 normalization on
target indegree, relu after each layer.

Strategy
--------
kNN in a 1-D metric means: after sorting nodes by density, every node's 4
nearest neighbours lie within +/-4 sorted positions. So the whole aggregation
matrix becomes a 9-diagonal *banded* matrix in sorted order. The device kernel:

  1. transposes X^T [Cin, N] tiles on the TensorEngine and indirect-DMA
     scatters node rows into a DRAM scratch in *sorted* order (per-core rank
     window of 2048 nodes + halo),
  2. computes A1 = Band @ X_s with small banded matmuls (TensorEngine,
     float32r = full-precision fp32 at 1 cycle/row),
  3. H^T = relu(W1^T A1^T + b1) dense matmuls (channel-major),
  4. T2^T = W2^T H^T, transposed back to node-major,
  5. out = relu(Band @ T2 + b2), indirect-DMA scattered to original node order.

Host does only O(N log N) index math on the 16 KB density array: argsort, band
weights w9[r, o] (including exact reference tie-breaking by (dist, orig index),
which also reproduces the reference's duplicate-density self-target quirk), and
expands them into the per-tile band matrices.

Sharding: 8 cores = 4 batches x 2 rank-halves. Core c handles batch c//2,
sorted ranks [ (c%2)*2048, (c%2)*2048+2048 ).
"""

import numpy as np

# ---------------------------------------------------------------- constants
B = 4
CIN = 256
CHID = 512
COUT = 256
H = W = 64
N = H * W            # 4096 nodes per batch
KNN = 4
BAND = 4             # kNN lies within +/-4 sorted positions
HALF = N // 2        # 2048 ranks per core
NT1 = 17             # A1/H/T2 tiles (rows r0-4 .. r0+2172)
NT2 = 16             # output tiles  (rows r0   .. r0+2048)
GATH_ROWS = (NT1 + 1) * 128  # 2304 gathered window rows (rank r0 - 8 + i)

_COMPILED = {}


# ---------------------------------------------------------------- host graph
def _build_band_weights(d_flat):
    """order [N], w9 [N, 9] f32: out_s[r] = sum_o w9[r, o+4] * g_s[r+o]."""
    order = np.argsort(d_flat, kind="stable")
    d_s = d_flat[order]

    offs = np.arange(-BAND, BAND + 1)
    ridx = np.arange(N)[:, None] + offs[None, :]
    valid = (ridx >= 0) & (ridx < N)
    ridx_c = np.clip(ridx, 0, N - 1)
    c = np.abs(d_s[ridx_c] - d_s[:, None]).astype(np.float32)
    c = np.where(valid, c, np.float32(np.inf))
    cand_j = np.where(valid, order[ridx_c], N)

    # reference = stable argsort over the full row: ties by smaller orig index.
    sel = np.lexsort((cand_j, c), axis=1)
    tgt_s = np.take_along_axis(ridx_c, sel[:, 1:KNN + 1], axis=1).reshape(-1)
    src_s = np.repeat(np.arange(N), KNN)

    deg = np.ones(N, dtype=np.float32)
    np.add.at(deg, tgt_s, np.float32(1.0))
    dinv = (np.float32(1.0) / np.sqrt(deg)).astype(np.float32)

    m = np.zeros((N, 9), dtype=np.float32)
    np.add.at(m, (tgt_s, src_s - tgt_s + BAND), np.float32(1.0))
    m[:, BAND] += 1.0  # self loops

    ro = np.arange(N)[:, None] + offs[None, :]
    rov = (ro >= 0) & (ro < N)
    w9 = m * dinv[:, None] * dinv[np.clip(ro, 0, N - 1)] * rov
    return order.astype(np.int32), w9.astype(np.float32)


def _host_graph(density_maps):
    """Per-core index/band tensors. Returns list of 8 dicts."""
    per_core = []
    for b in range(B):
        d = np.asarray(density_maps[b]).reshape(N).astype(np.float32)
        order, w9g = _build_band_weights(d)
        rank = np.empty(N, dtype=np.int64)
        rank[order] = np.arange(N)
        for half in range(2):
            r0 = half * HALF

            # gather index: local window row i (rank r0 - 8 + i) -> orig node.
            # Out-of-range ranks clip to node 0 (finite data; w9 rows are 0 there).
            gi = np.arange(GATH_ROWS) + (r0 - 8)
            gsrc = np.where((gi >= 0) & (gi < N), order[np.clip(gi, 0, N - 1)], 0)
            gidx = np.tile(gsrc.reshape(GATH_ROWS // 16, 16).T.astype(np.int16), (8, 1)).copy()  # [128, 144]

            # w9 rows for this core's window, zero outside usable range
            # w9_dev[i] = w9 at rank (r0 - 4 + i), i in [0, NT1*128)
            w9_dev = np.zeros((NT1 * 128, 9), dtype=np.float32)
            g = np.arange(NT1 * 128) + (r0 - 4)
            ok = (g >= 0) & (g < N) & (g < r0 + HALF + 4)
            w9_dev[ok] = w9g[g[ok]]

            # band matrices bandT[k, q, r]: k<17 -> L1 tile (out rows r0-4+128k+r),
            # k>=17 -> L2 tile (out rows r0+128(k-17)+r). value = w9row[q - r].
            bandT = np.zeros((NT1 + NT2, 136, 128), dtype=np.float32)
            qq = np.arange(136)[:, None]          # window position
            rr = np.arange(128)[None, :]          # out row within tile
            dd = qq - rr                          # w9 column (o + 4)
            okd = (dd >= 0) & (dd < 9)
            dd_c = np.clip(dd, 0, 8)
            rr_b = np.broadcast_to(rr, (136, 128))
            for k in range(NT1 + NT2):
                base = 128 * k if k < NT1 else 4 + 128 * (k - NT1)
                rows = w9_dev[base + np.arange(128)]          # [128, 9]
                bandT[k] = np.where(okd, rows[rr_b, dd_c], 0.0)

            # output scatter: flat i (rank r0 + i) -> orig node index
            osrc = order[r0 + np.arange(NT2 * 128)]
            oidx = np.tile(osrc.reshape(NT2 * 128 // 16, 16).T.astype(np.int16), (8, 1)).copy()  # [128, 128]

            per_core.append(dict(gidx=gidx, oidx=oidx,
                                 bandT=np.ascontiguousarray(bandT.transpose(1, 0, 2)),
                                 order=order, rank=rank))
    return per_core


# ---------------------------------------------------------------- device IR
def build_nc():
    import concourse.bass as bass
    import concourse.bacc as bacc
    import concourse.mybir as mybir
    from concourse.tile import TileContext

    F32 = mybir.dt.float32
    F32R = mybir.dt.float32r
    I32 = mybir.dt.int32
    I16 = mybir.dt.int16
    NR = NT1 + NT2

    nc = bacc.Bacc()
    xT = nc.dram_tensor("xT", [CIN, N], F32R, kind="ExternalInput")
    w1 = nc.dram_tensor("w1", [CIN, CHID], F32R, kind="ExternalInput")
    w2 = nc.dram_tensor("w2", [CHID, COUT], F32R, kind="ExternalInput")
    b1 = nc.dram_tensor("b1", [CHID], F32, kind="ExternalInput")
    b2rep = nc.dram_tensor("b2rep", [128, COUT], F32, kind="ExternalInput")
    ident = nc.dram_tensor("ident", [128, 128], F32R, kind="ExternalInput")
    bandT = nc.dram_tensor("bandT", [136, NR, 128], F32R, kind="ExternalInput")
    gidx = nc.dram_tensor("gidx", [128, GATH_ROWS // 16], I16, kind="ExternalInput")
    oidx = nc.dram_tensor("oidx", [128, NT2 * 128 // 16], I16, kind="ExternalInput")
    out_nodes = nc.dram_tensor("out_nodes", [N, COUT], F32, kind="ExternalOutput")
    xpose = nc.dram_tensor("xpose", [N, CIN], F32R, kind="Internal")

    NJ = N // 128  # 32 node-column tiles of xT

    with TileContext(nc) as tc:
        with (
            tc.tile_pool(name="const", bufs=1) as cpool,
            tc.tile_pool(name="big", bufs=1) as big,
            tc.tile_pool(name="stream", bufs=3) as sp,
            tc.tile_pool(name="psum", bufs=2, space="PSUM") as pp,
        ):
            ident_sb = cpool.tile([128, 128], F32R)
            nc.sync.dma_start(ident_sb, ident[:, :])
            b2_sb = cpool.tile([128, COUT], F32)
            nc.scalar.dma_start(b2_sb, b2rep[:, :])
            zero_sb = cpool.tile([128, CIN], F32)
            nc.gpsimd.memset(zero_sb, 0.0)

            w1_sb = cpool.tile([128, 2, CHID], F32R)   # [k-part, k-chunk, m]
            nc.scalar.dma_start(w1_sb, w1.rearrange("(c p) m -> p c m", p=128))
            w2_sb = cpool.tile([128, 4, COUT], F32R)
            nc.scalar.dma_start(w2_sb, w2.rearrange("(c p) m -> p c m", p=128))
            b1_sb = cpool.tile([128, 4], F32)
            nc.scalar.dma_start(b1_sb, b1.rearrange("(c p) -> p c", p=128))
            gidx_sb = cpool.tile([128, GATH_ROWS // 16], I16)
            nc.scalar.dma_start(gidx_sb, gidx[:, :])
            oidx_sb = cpool.tile([128, NT2 * 128 // 16], I16)
            nc.scalar.dma_start(oidx_sb, oidx[:, :])

            # all band matrices in two DMAs: [q-part, region, r]
            bandA_sb = cpool.tile([128, NR, 128], F32R)
            nc.scalar.dma_start(bandA_sb, bandT[0:128, :, :])
            bandB_sb = cpool.tile([8, NR, 128], F32R)
            nc.scalar.dma_start(bandB_sb, bandT[128:136, :, :])

            # ---------------- phase X: transpose X^T tiles into node-major DRAM,
            # then one dma_gather pulls the sorted window into SBUF.
            for jh in range(NJ // 4):
                xt_sb = sp.tile([128, 512], F32R, tag="xt")
                nc.sync.dma_start(xt_sb, xT[0:128, 512 * jh:512 * (jh + 1)])
                xt_sb2 = sp.tile([128, 512], F32R, tag="xt2")
                nc.sync.dma_start(xt_sb2, xT[128:256, 512 * jh:512 * (jh + 1)])
                for j4 in range(4):
                    jt = 4 * jh + j4
                    tp = pp.tile([128, 256], F32R, tag="tp", space="PSUM")
                    nc.tensor.transpose(tp[:, 0:128], xt_sb[:, 128 * j4:128 * (j4 + 1)], ident_sb)
                    nc.tensor.transpose(tp[:, 128:256], xt_sb2[:, 128 * j4:128 * (j4 + 1)], ident_sb)
                    xn = sp.tile([128, CIN], F32R, tag="xn")
                    nc.vector.tensor_copy(xn, tp)
                    nc.sync.dma_start(xpose[128 * jt:128 * (jt + 1), :], xn)

            # zero the output accumulator (scatter-add target); scalar ring,
            # overlaps the gather/compute phases
            zero_big = cpool.tile([128, 1024], F32)
            nc.gpsimd.memset(zero_big, 0.0)
            for r in range(0, N, 512):
                nc.scalar.dma_start(
                    out_nodes[r:r + 512, :].rearrange("(a b) c -> a (b c)", b=4),
                    zero_big[:, :])

            gath = big.tile([128, NT1 + 1, CIN], F32R)
            nc.gpsimd.dma_gather(gath[:, 0:9, :], xpose[:, :], gidx_sb[:, 0:72],
                                 9 * 128, 9 * 128, CIN, single_packet=False)
            nc.gpsimd.dma_gather(gath[:, 9:18, :], xpose[:, :], gidx_sb[:, 72:144],
                                 9 * 128, 9 * 128, CIN, single_packet=False)

            # ---------------- L1 aggregation: A1 = Band1 @ X_s (node-major psum),
            # then transpose to A1^T (cin-major) for the dense matmul.
            a1T = big.tile([128, 2, NT1 * 128], F32R)   # A1^T, cin-chunk major
            for t in range(NT1):
                psA = pp.tile([128, CIN], F32, tag="agg", space="PSUM")
                nc.tensor.matmul(psA, lhsT=bandA_sb[:, t, :], rhs=gath[:, t, :],
                                 start=True, stop=False)
                nc.tensor.matmul(psA, lhsT=bandB_sb[:, t, :],
                                 rhs=gath[0:8, t + 1, :],
                                 start=False, stop=True)
                a1_sb = sp.tile([128, CIN], F32R, tag="a1")
                nc.vector.tensor_copy(a1_sb, psA)
                for cb in range(2):
                    tpa = pp.tile([128, 128], F32R, tag="tp", space="PSUM")
                    nc.tensor.transpose(tpa, a1_sb[:, 128 * cb:128 * (cb + 1)], ident_sb)
                    nc.scalar.activation(a1T[:, cb, 128 * t:128 * t + 128], tpa,
                                         mybir.ActivationFunctionType.Copy)

            # ---------------- L1 dense: H^T = relu(W1^T A1^T + b1)  (chid-major)
            NODES = NT1 * 128
            blocks = [(i, min(i + 448, NODES)) for i in range(0, NODES, 448)]
            hT = big.tile([128, 4, NODES], F32R)
            for lo, hi in blocks:
                for mb in range(4):
                    psH = pp.tile([128, 448], F32, tag="dense", space="PSUM")
                    for kb in range(2):
                        nc.tensor.matmul(
                            psH[:, 0:hi - lo],
                            lhsT=w1_sb[:, kb, 128 * mb:128 * (mb + 1)],
                            rhs=a1T[:, kb, lo:hi],
                            start=(kb == 0), stop=(kb == 1))
                    nc.scalar.activation(
                        hT[:, mb, lo:hi], psH[:, 0:hi - lo],
                        mybir.ActivationFunctionType.Relu,
                        bias=b1_sb[:, mb:mb + 1], scale=1.0)

            # ---------------- L2 dense: T2 = H W2, node-major directly
            # lhsT = H^T slice [chid-chunk, 128 nodes], rhs = W2 chunk
            t2n = big.tile([128, NT1, COUT], F32R)
            for t in range(NT1):
                psT = pp.tile([128, COUT], F32, tag="agg", space="PSUM")
                for kb in range(4):
                    nc.tensor.matmul(
                        psT,
                        lhsT=hT[:, kb, 128 * t:128 * t + 128],
                        rhs=w2_sb[:, kb, :],
                        start=(kb == 0), stop=(kb == 3))
                nc.scalar.activation(t2n[:, t, :], psT,
                                     mybir.ActivationFunctionType.Copy)

            # ---------------- L2 aggregation + b2 (as K=1 matmul) + relu + scatter
            out_all = big.tile([128, NT2, COUT], F32)
            for t in range(NT2):
                psO = pp.tile([128, COUT], F32, tag="agg", space="PSUM")
                nc.tensor.matmul(psO, lhsT=bandA_sb[:, NT1 + t, :],
                                 rhs=t2n[:, t, :], start=True, stop=False)
                nc.tensor.matmul(psO, lhsT=bandB_sb[:, NT1 + t, :],
                                 rhs=t2n[0:8, t + 1, :], start=False, stop=True)
                nc.vector.tensor_tensor(out=out_all[:, t, :], in0=psO, in1=b2_sb,
                                        op=mybir.AluOpType.add)
                nc.scalar.activation(out_all[:, t, :], out_all[:, t, :],
                                     mybir.ActivationFunctionType.Relu)
                if t % 4 == 3:
                    q = t // 4
                    nc.gpsimd.dma_scatter_add(
                        out_nodes[:, :], out_all[:, 4 * q:4 * q + 4, :],
                        oidx_sb[:, 32 * q:32 * q + 32], 512, 512, COUT,
                        single_packet=False)

    nc.compile()
    return nc


def _round_f32r(a):
    bits = np.ascontiguousarray(a, dtype=np.float32).view(np.uint32)
    r = ((bits.astype(np.uint64) + 0x800) & np.uint64(0xFFFFF000)).astype(np.uint32)
    return r.view(np.float32)


def make_in_maps(density_maps, feature_maps, W1, b1, W2, b2):
    graph = _host_graph(density_maps)
    fm = np.ascontiguousarray(np.asarray(feature_maps, dtype=np.float32))
    W1 = np.ascontiguousarray(np.asarray(W1, dtype=np.float32))
    W2 = np.ascontiguousarray(np.asarray(W2, dtype=np.float32))
    b1 = np.ascontiguousarray(np.asarray(b1, dtype=np.float32))
    b2r = np.broadcast_to(np.asarray(b2, dtype=np.float32), (128, COUT)).copy()
    in_maps = []
    for c in range(8):
        g = graph[c]
        in_maps.append({
            "xT": fm[c // 2].reshape(CIN, N),
            "w1": _round_f32r(W1), "w2": _round_f32r(W2), "b1": b1,
            "b2rep": b2r, "ident": np.eye(128, dtype=np.float32),
            "bandT": _round_f32r(g["bandT"]), "gidx": g["gidx"], "oidx": g["oidx"],
        })
    return in_maps, graph


def kernel(density_maps, feature_maps, W1, b1, W2, b2):
    from concourse.bass_utils import run_bass_kernel_spmd

    if "nc" not in _COMPILED:
        _COMPILED["nc"] = build_nc()
    nc = _COMPILED["nc"]

    in_maps, graph = make_in_maps(density_maps, feature_maps, W1, b1, W2, b2)
    res = run_bass_kernel_spmd(nc, in_maps, core_ids=list(range(8)))

    out = np.empty((B, N, COUT), dtype=np.float32)
    for b in range(B):
        o0 = res.results[2 * b]["out_nodes"]
        o1 = res.results[2 * b + 1]["out_nodes"]
        mask = (graph[2 * b]["rank"] < HALF)[:, None]
        out[b] = np.where(mask, o0, o1)
    return np.ascontiguousarray(
        out.reshape(B, H, W, COUT).transpose(0, 3, 1, 2)).astype(np.float32)


# revision 16
# speedup vs baseline: 1.4605x; 1.1277x over previous
"""Trainium2 Bass kernel for DensityGCNProcessor.

Model: 2-layer GCN over a per-sample kNN graph built from 1-D density values
(K=4 nearest by |density_i - density_j|), symmetric deg^-1/2 normalization on
target indegree, relu after each layer.

Strategy
--------
kNN in a 1-D metric means: after sorting nodes by density, every node's 4
nearest neighbours lie within +/-4 sorted positions. So the whole aggregation
matrix becomes a 9-diagonal *banded* matrix in sorted order. The device kernel:

  1. transposes X^T [Cin, N] tiles on the TensorEngine and indirect-DMA
     scatters node rows into a DRAM scratch in *sorted* order (per-core rank
     window of 2048 nodes + halo),
  2. computes A1 = Band @ X_s with small banded matmuls (TensorEngine,
     float32r = full-precision fp32 at 1 cycle/row),
  3. H^T = relu(W1^T A1^T + b1) dense matmuls (channel-major),
  4. T2^T = W2^T H^T, transposed back to node-major,
  5. out = relu(Band @ T2 + b2), indirect-DMA scattered to original node order.

Host does only O(N log N) index math on the 16 KB density array: argsort, band
weights w9[r, o] (including exact reference tie-breaking by (dist, orig index),
which also reproduces the reference's duplicate-density self-target quirk), and
expands them into the per-tile band matrices.

Sharding: 8 cores = 4 batches x 2 rank-halves. Core c handles batch c//2,
sorted ranks [ (c%2)*2048, (c%2)*2048+2048 ).
"""

import numpy as np

# ---------------------------------------------------------------- constants
B = 4
CIN = 256
CHID = 512
COUT = 256
H = W = 64
N = H * W            # 4096 nodes per batch
KNN = 4
BAND = 4             # kNN lies within +/-4 sorted positions
HALF = N // 2        # 2048 ranks per core
NT1 = 17             # A1/H/T2 tiles (rows r0-4 .. r0+2172)
NT2 = 16             # output tiles  (rows r0   .. r0+2048)
GATH_ROWS = (NT1 + 1) * 128  # 2304 gathered window rows (rank r0 - 8 + i)

_COMPILED = {}


# ---------------------------------------------------------------- host graph
def _build_band_weights(d_flat):
    """order [N], w9 [N, 9] f32: out_s[r] = sum_o w9[r, o+4] * g_s[r+o]."""
    order = np.argsort(d_flat, kind="stable")
    d_s = d_flat[order]

    offs = np.arange(-BAND, BAND + 1)
    ridx = np.arange(N)[:, None] + offs[None, :]
    valid = (ridx >= 0) & (ridx < N)
    ridx_c = np.clip(ridx, 0, N - 1)
    c = np.abs(d_s[ridx_c] - d_s[:, None]).astype(np.float32)
    c = np.where(valid, c, np.float32(np.inf))
    cand_j = np.where(valid, order[ridx_c], N)

    # reference = stable argsort over the full row: ties by smaller orig index.
    sel = np.lexsort((cand_j, c), axis=1)
    tgt_s = np.take_along_axis(ridx_c, sel[:, 1:KNN + 1], axis=1).reshape(-1)
    src_s = np.repeat(np.arange(N), KNN)

    deg = np.ones(N, dtype=np.float32)
    np.add.at(deg, tgt_s, np.float32(1.0))
    dinv = (np.float32(1.0) / np.sqrt(deg)).astype(np.float32)

    m = np.zeros((N, 9), dtype=np.float32)
    np.add.at(m, (tgt_s, src_s - tgt_s + BAND), np.float32(1.0))
    m[:, BAND] += 1.0  # self loops

    ro = np.arange(N)[:, None] + offs[None, :]
    rov = (ro >= 0) & (ro < N)
    w9 = m * dinv[:, None] * dinv[np.clip(ro, 0, N - 1)] * rov
    return order.astype(np.int32), w9.astype(np.float32)


def _host_graph(density_maps):
    """Per-core index/band tensors. Returns list of 8 dicts."""
    per_core = []
    for b in range(B):
        d = np.asarray(density_maps[b]).reshape(N).astype(np.float32)
        order, w9g = _build_band_weights(d)
        rank = np.empty(N, dtype=np.int64)
        rank[order] = np.arange(N)
        for half in range(2):
            r0 = half * HALF

            # gather index: local window row i (rank r0 - 8 + i) -> orig node.
            # Out-of-range ranks clip to node 0 (finite data; w9 rows are 0 there).
            gi = np.arange(GATH_ROWS) + (r0 - 8)
            gsrc = np.where((gi >= 0) & (gi < N), order[np.clip(gi, 0, N - 1)], 0)
            gidx = np.tile(gsrc.reshape(GATH_ROWS // 16, 16).T.astype(np.int16), (8, 1)).copy()  # [128, 144]

            # w9 rows for this core's window, zero outside usable range
            # w9_dev[i] = w9 at rank (r0 - 4 + i), i in [0, NT1*128)
            w9_dev = np.zeros((NT1 * 128, 9), dtype=np.float32)
            g = np.arange(NT1 * 128) + (r0 - 4)
            ok = (g >= 0) & (g < N) & (g < r0 + HALF + 4)
            w9_dev[ok] = w9g[g[ok]]

            # band matrices bandT[k, q, r]: k<17 -> L1 tile (out rows r0-4+128k+r),
            # k>=17 -> L2 tile (out rows r0+128(k-17)+r). value = w9row[q - r].
            bandT = np.zeros((NT1 + NT2, 136, 128), dtype=np.float32)
            qq = np.arange(136)[:, None]          # window position
            rr = np.arange(128)[None, :]          # out row within tile
            dd = qq - rr                          # w9 column (o + 4)
            okd = (dd >= 0) & (dd < 9)
            dd_c = np.clip(dd, 0, 8)
            rr_b = np.broadcast_to(rr, (136, 128))
            for k in range(NT1 + NT2):
                base = 128 * k if k < NT1 else 4 + 128 * (k - NT1)
                rows = w9_dev[base + np.arange(128)]          # [128, 9]
                bandT[k] = np.where(okd, rows[rr_b, dd_c], 0.0)

            # output scatter: flat i (rank r0 + i) -> orig node index
            osrc = order[r0 + np.arange(NT2 * 128)]
            oidx = np.tile(osrc.reshape(NT2 * 128 // 16, 16).T.astype(np.int16), (8, 1)).copy()  # [128, 128]

            per_core.append(dict(gidx=gidx, oidx=oidx,
                                 bandT=np.ascontiguousarray(bandT.transpose(1, 0, 2)),
                                 order=order, rank=rank))
    return per_core


# ---------------------------------------------------------------- device IR
def build_nc():
    import concourse.bass as bass
    import concourse.bacc as bacc
    import concourse.mybir as mybir
    from concourse.tile import TileContext

    F32 = mybir.dt.float32
    F32R = mybir.dt.float32r
    I32 = mybir.dt.int32
    I16 = mybir.dt.int16
    NR = NT1 + NT2

    nc = bacc.Bacc()
    xT = nc.dram_tensor("xT", [CIN, N], F32R, kind="ExternalInput")
    w1 = nc.dram_tensor("w1", [CIN, CHID], F32R, kind="ExternalInput")
    w2 = nc.dram_tensor("w2", [CHID, COUT], F32R, kind="ExternalInput")
    b1 = nc.dram_tensor("b1", [CHID], F32, kind="ExternalInput")
    b2rep = nc.dram_tensor("b2rep", [128, COUT], F32, kind="ExternalInput")
    ident = nc.dram_tensor("ident", [128, 128], F32R, kind="ExternalInput")
    bandT = nc.dram_tensor("bandT", [136, NR, 128], F32R, kind="ExternalInput")
    gidx = nc.dram_tensor("gidx", [128, GATH_ROWS // 16], I16, kind="ExternalInput")
    oidx = nc.dram_tensor("oidx", [128, NT2 * 128 // 16], I16, kind="ExternalInput")
    out_nodes = nc.dram_tensor("out_nodes", [N, COUT], F32, kind="ExternalOutput")
    xpose = nc.dram_tensor("xpose", [N, CIN], F32R, kind="Internal")

    NJ = N // 128  # 32 node-column tiles of xT

    with TileContext(nc) as tc:
        with (
            tc.tile_pool(name="const", bufs=1) as cpool,
            tc.tile_pool(name="big", bufs=1) as big,
            tc.tile_pool(name="stream", bufs=3) as sp,
            tc.tile_pool(name="psum", bufs=2, space="PSUM") as pp,
        ):
            ident_sb = cpool.tile([128, 128], F32R)
            nc.sync.dma_start(ident_sb, ident[:, :])
            b2_sb = cpool.tile([128, COUT], F32)
            nc.scalar.dma_start(b2_sb, b2rep[:, :])
            zero_sb = cpool.tile([128, CIN], F32)
            nc.gpsimd.memset(zero_sb, 0.0)

            w1_sb = cpool.tile([128, 2, CHID], F32R)   # [k-part, k-chunk, m]
            nc.scalar.dma_start(w1_sb, w1.rearrange("(c p) m -> p c m", p=128))
            w2_sb = cpool.tile([128, 4, COUT], F32R)
            nc.scalar.dma_start(w2_sb, w2.rearrange("(c p) m -> p c m", p=128))
            b1_sb = cpool.tile([128, 4], F32)
            nc.scalar.dma_start(b1_sb, b1.rearrange("(c p) -> p c", p=128))
            gidx_sb = cpool.tile([128, GATH_ROWS // 16], I16)
            nc.scalar.dma_start(gidx_sb, gidx[:, :])
            oidx_sb = cpool.tile([128, NT2 * 128 // 16], I16)
            nc.scalar.dma_start(oidx_sb, oidx[:, :])

            # all band matrices in two DMAs: [q-part, region, r]
            bandA_sb = cpool.tile([128, NR, 128], F32R)
            nc.scalar.dma_start(bandA_sb, bandT[0:128, :, :])
            bandB_sb = cpool.tile([8, NR, 128], F32R)
            nc.scalar.dma_start(bandB_sb, bandT[128:136, :, :])

            # ---------------- phase X: transpose X^T tiles into node-major DRAM,
            # then one dma_gather pulls the sorted window into SBUF.
            for jh in range(NJ // 4):
                xt_sb = sp.tile([128, 512], F32R, tag="xt")
                nc.sync.dma_start(xt_sb, xT[0:128, 512 * jh:512 * (jh + 1)])
                xt_sb2 = sp.tile([128, 512], F32R, tag="xt2")
                nc.sync.dma_start(xt_sb2, xT[128:256, 512 * jh:512 * (jh + 1)])
                xnB = sp.tile([128, 4, CIN], F32R, tag="xn")
                for jp in range(2):
                    tp = pp.tile([128, 512], F32R, tag="tp", space="PSUM")
                    for jj in range(2):
                        j4 = 2 * jp + jj
                        nc.tensor.transpose(tp[:, 256 * jj:256 * jj + 128],
                                            xt_sb[:, 128 * j4:128 * (j4 + 1)], ident_sb)
                        nc.tensor.transpose(tp[:, 256 * jj + 128:256 * jj + 256],
                                            xt_sb2[:, 128 * j4:128 * (j4 + 1)], ident_sb)
                    nc.vector.tensor_copy(xnB[:, 2 * jp:2 * jp + 2, :], tp)
                nc.sync.dma_start(xpose[512 * jh:512 * (jh + 1), :]
                                  .rearrange("(j p) c -> p j c", p=128), xnB)

            # zero the output accumulator (scatter-add target); scalar ring,
            # overlaps the gather/compute phases
            zero_big = cpool.tile([128, 1024], F32)
            nc.gpsimd.memset(zero_big, 0.0)
            for r in range(0, N, 512):
                nc.scalar.dma_start(
                    out_nodes[r:r + 512, :].rearrange("(a b) c -> a (b c)", b=4),
                    zero_big[:, :])

            gath = big.tile([128, NT1 + 1, CIN], F32R)
            nc.gpsimd.dma_gather(gath[:, 0:9, :], xpose[:, :], gidx_sb[:, 0:72],
                                 9 * 128, 9 * 128, CIN, single_packet=False)
            nc.gpsimd.dma_gather(gath[:, 9:18, :], xpose[:, :], gidx_sb[:, 72:144],
                                 9 * 128, 9 * 128, CIN, single_packet=False)

            # ---------------- L1 aggregation: A1 = Band1 @ X_s (node-major psum),
            # then transpose to A1^T (cin-major) for the dense matmul.
            a1T = big.tile([128, 2, NT1 * 128], F32R)   # A1^T, cin-chunk major
            for t in range(NT1):
                psA = pp.tile([128, CIN], F32, tag="agg", space="PSUM")
                nc.tensor.matmul(psA, lhsT=bandA_sb[:, t, :], rhs=gath[:, t, :],
                                 start=True, stop=False)
                nc.tensor.matmul(psA, lhsT=bandB_sb[:, t, :],
                                 rhs=gath[0:8, t + 1, :],
                                 start=False, stop=True)
                a1_sb = sp.tile([128, CIN], F32R, tag="a1")
                nc.vector.tensor_copy(a1_sb, psA)
                for cb in range(2):
                    tpa = pp.tile([128, 128], F32R, tag="tp", space="PSUM")
                    nc.tensor.transpose(tpa, a1_sb[:, 128 * cb:128 * (cb + 1)], ident_sb)
                    nc.vector.tensor_copy(a1T[:, cb, 128 * t:128 * t + 128], tpa)

            # ---------------- L1 dense: H^T = relu(W1^T A1^T + b1)  (chid-major)
            NODES = NT1 * 128
            blocks = [(i, min(i + 448, NODES)) for i in range(0, NODES, 448)]
            hT = big.tile([128, 4, NODES], F32R)
            for lo, hi in blocks:
                for mb in range(4):
                    psH = pp.tile([128, 448], F32, tag="dense", space="PSUM")
                    for kb in range(2):
                        nc.tensor.matmul(
                            psH[:, 0:hi - lo],
                            lhsT=w1_sb[:, kb, 128 * mb:128 * (mb + 1)],
                            rhs=a1T[:, kb, lo:hi],
                            start=(kb == 0), stop=(kb == 1))
                    nc.scalar.activation(
                        hT[:, mb, lo:hi], psH[:, 0:hi - lo],
                        mybir.ActivationFunctionType.Relu,
                        bias=b1_sb[:, mb:mb + 1], scale=1.0)

            # ---------------- L2 dense: T2 = H W2, node-major directly
            # lhsT = H^T slice [chid-chunk, 128 nodes], rhs = W2 chunk
            t2n = big.tile([128, NT1, COUT], F32R)
            for t in range(NT1):
                psT = pp.tile([128, COUT], F32, tag="agg", space="PSUM")
                for kb in range(4):
                    nc.tensor.matmul(
                        psT,
                        lhsT=hT[:, kb, 128 * t:128 * t + 128],
                        rhs=w2_sb[:, kb, :],
                        start=(kb == 0), stop=(kb == 3))
                nc.scalar.activation(t2n[:, t, :], psT,
                                     mybir.ActivationFunctionType.Copy)

            # ---------------- L2 aggregation + b2 (as K=1 matmul) + relu + scatter
            out_all = big.tile([128, NT2, COUT], F32)
            for t in range(NT2):
                psO = pp.tile([128, COUT], F32, tag="agg", space="PSUM")
                nc.tensor.matmul(psO, lhsT=bandA_sb[:, NT1 + t, :],
                                 rhs=t2n[:, t, :], start=True, stop=False)
                nc.tensor.matmul(psO, lhsT=bandB_sb[:, NT1 + t, :],
                                 rhs=t2n[0:8, t + 1, :], start=False, stop=True)
                nc.vector.tensor_tensor(out=out_all[:, t, :], in0=psO, in1=b2_sb,
                                        op=mybir.AluOpType.add)
                nc.scalar.activation(out_all[:, t, :], out_all[:, t, :],
                                     mybir.ActivationFunctionType.Relu)
                if t % 8 == 7:
                    q = t // 8
                    nc.gpsimd.dma_scatter_add(
                        out_nodes[:, :], out_all[:, 8 * q:8 * q + 8, :],
                        oidx_sb[:, 64 * q:64 * q + 64], 1024, 1024, COUT,
                        single_packet=False)

    nc.compile()
    return nc


def _round_f32r(a):
    bits = np.ascontiguousarray(a, dtype=np.float32).view(np.uint32)
    r = ((bits.astype(np.uint64) + 0x800) & np.uint64(0xFFFFF000)).astype(np.uint32)
    return r.view(np.float32)


def make_in_maps(density_maps, feature_maps, W1, b1, W2, b2):
    graph = _host_graph(density_maps)
    fm = np.ascontiguousarray(np.asarray(feature_maps, dtype=np.float32))
    W1 = np.ascontiguousarray(np.asarray(W1, dtype=np.float32))
    W2 = np.ascontiguousarray(np.asarray(W2, dtype=np.float32))
    b1 = np.ascontiguousarray(np.asarray(b1, dtype=np.float32))
    b2r = np.broadcast_to(np.asarray(b2, dtype=np.float32), (128, COUT)).copy()
    in_maps = []
    for c in range(8):
        g = graph[c]
        in_maps.append({
            "xT": fm[c // 2].reshape(CIN, N),
            "w1": _round_f32r(W1), "w2": _round_f32r(W2), "b1": b1,
            "b2rep": b2r, "ident": np.eye(128, dtype=np.float32),
            "bandT": _round_f32r(g["bandT"]), "gidx": g["gidx"], "oidx": g["oidx"],
        })
    return in_maps, graph


def kernel(density_maps, feature_maps, W1, b1, W2, b2):
    from concourse.bass_utils import run_bass_kernel_spmd

    if "nc" not in _COMPILED:
        _COMPILED["nc"] = build_nc()
    nc = _COMPILED["nc"]

    in_maps, graph = make_in_maps(density_maps, feature_maps, W1, b1, W2, b2)
    res = run_bass_kernel_spmd(nc, in_maps, core_ids=list(range(8)))

    out = np.empty((B, N, COUT), dtype=np.float32)
    for b in range(B):
        o0 = res.results[2 * b]["out_nodes"]
        o1 = res.results[2 * b + 1]["out_nodes"]
        mask = (graph[2 * b]["rank"] < HALF)[:, None]
        out[b] = np.where(mask, o0, o1)
    return np.ascontiguousarray(
        out.reshape(B, H, W, COUT).transpose(0, 3, 1, 2)).astype(np.float32)
